# revision 23
# baseline (speedup 1.0000x reference)
"""CTC loss (mean reduction) on 8 Trainium2 NeuronCores.

Data-parallel over batch: 4 utterances per core, one partition each, with the
S=257 extended-label states on the free axis. The lattice DP runs t-major in
the linear-probability domain in fp32:

    A_t[s] = (A_{t-1}[s] + e^{-g} A_{t-1}[s-1] + m3[s] e^{-2g} A_{t-1}[s-2]) * p_t[s]

Emissions are shipped as 4-bit log2-quantized codes (two time steps packed
per byte, nibble 0 = exact zero, nibble n = 2^(n-8)). Since all 129 even
states carry the blank label, each frame ships only 128 label values plus
one shared blank value; the device decodes (vector: nibble split, u8->f32,
zero-masks; activation: exp) and expands to the full 257-state planes with
two strided tensor_copies (stride-2 writes, stride-0 broadcast reads).
Range control needs no host-side DP oracle:
  * a per-utterance constant shift puts emissions on the 4-bit grid;
  * an exact per-utterance "tilt" e^{-g*s} (g fitted from the advance rate
    sl/il) is folded into the transition weights so the renormed lattice
    tracks the answer diagonal;
  * every RN=8 steps the alpha row sum (accum_out on the step itself) is
    reciprocal'd and folded into a following step's emission multiply
    (deferred renorm); factors are shipped back and log-summed on host.
Past t=il-1 each utterance is padded with blank=1/labels=0: odd states die
in one step, every even state then freezes (no skip transitions into even
states), and A[sl] holds alpha[sl] + e^{-g} alpha[sl-1] -- the CTC
log-likelihood lands in the final alpha row.

Engines: sync issues all DMAs (fully unrolled, static), activation does the
two exp's per chunk, vector does decode + DP with a hardware Fori loop over
chunk pairs (32 time steps per chunk), double-buffered via semaphores.
"""

import math
import os

import jax

# run_bass_kernel_spmd rebuilds a fresh jax.jit each call; with the
# persistent compilation cache enabled the identical-HLO wrapper (which
# embeds the compiled NEFF) deserializes from disk instead of recompiling,
# cutting ~150ms/call of XLA compile off the dispatch path.
try:
    _cache_dir = os.path.join(
        os.environ.get("XDG_CACHE_HOME", "/tmp"), "jax_ctc_cache")
    os.makedirs(_cache_dir, exist_ok=True)
    jax.config.update("jax_compilation_cache_dir", _cache_dir)
    jax.config.update("jax_persistent_cache_min_compile_time_secs", 0.0)
    jax.config.update("jax_persistent_cache_min_entry_size_bytes", -1)
except Exception:
    pass

import numpy as np

import concourse.bass as bass
import concourse.mybir as mybir
from concourse.bass_utils import run_bass_kernel_spmd

B, T, C, U = 32, 1000, 1024, 128
S = 2 * U + 1            # 257 extended states
NCORES = 8
BPC = B // NCORES        # 4 utterances per core
CT = 32                  # time steps per chunk
NCH = 33                 # chunks (odd; chunk 0 unrolled, rest looped in pairs)
TP = NCH * CT            # padded time 1056
NIT = (NCH - 1) // 2     # Fori iterations
RN = 8                   # renorm period (steps)
HW2 = (CT // 2) * S      # half-chunk decoded plane width 4112
LW = (S - 1) // 2        # distinct labels per frame 128
DW = (CT // 2) * LW + CT // 2   # packed chunk bytes 2064 (labels + shared blank)
OB = 260                 # outd column where renorm factors start
OUTW = OB + 4 + 8 * NIT  # 392
QB = 8.0                 # 4-bit grid bias: nibble n -> 2^(n-QB), 0 -> 0
LN2 = math.log(2.0)
F32 = mybir.dt.float32
U8 = mybir.dt.uint8
OP = mybir.AluOpType
AF = mybir.ActivationFunctionType
AX = mybir.AxisListType
# tilt fit: g = polyval(GCO, sl/il), calibrated on the input distribution
GCO = (4.0775, -6.8982, 3.1779)


def _build_nc(detect_races=True):
    nc = bass.Bass(detect_race_conditions=detect_races)
    pt = nc.declare_dram_parameter("pt", [BPC, NCH * DW], U8, isOutput=False)
    m3eg = nc.declare_dram_parameter("m3eg", [BPC, S + 1], F32, isOutput=False)
    outd = nc.declare_dram_parameter("outd", [BPC, OUTW], F32, isOutput=True)

    from contextlib import ExitStack

    with ExitStack() as ctx:
        s_in = ctx.enter_context(nc.semaphore("s_in"))
        s_upc = ctx.enter_context(nc.semaphore("s_upc"))
        s_dv = ctx.enter_context(nc.semaphore("s_dv"))
        s_da = ctx.enter_context(nc.semaphore("s_da"))
        s_free = ctx.enter_context(nc.semaphore("s_free"))
        s_out = ctx.enter_context(nc.semaphore("s_out"))

        def sb(name, shape, dt):
            return ctx.enter_context(nc.sbuf_tensor(name, shape, dt))

        xqA = sb("xqA", [BPC, DW], U8)
        xqB = sb("xqB", [BPC, DW], U8)
        n8l = sb("n8l", [BPC, DW], U8)
        n8h = sb("n8h", [BPC, DW], U8)
        nlo = sb("nlo", [BPC, DW], F32)
        nhi = sb("nhi", [BPC, DW], F32)
        elo = sb("elo", [BPC, DW], F32)
        ehi = sb("ehi", [BPC, DW], F32)
        mlo = sb("mlo", [BPC, DW], F32)
        mhi = sb("mhi", [BPC, DW], F32)
        pSL = sb("pSL", [BPC, DW], F32)
        pSH = sb("pSH", [BPC, DW], F32)
        pLo = sb("pLo", [BPC, HW2], F32)
        pHi = sb("pHi", [BPC, HW2], F32)
        bcon = sb("bcon", [BPC, 1], F32)
        M3 = sb("M3", [BPC, S + 1], F32)
        AE = sb("AE", [BPC, S + 2], F32)
        AO = sb("AO", [BPC, S + 2], F32)
        s1t = sb("s1t", [BPC, S], F32)
        a3t = sb("a3t", [BPC, S], F32)
        s2t = sb("s2t", [BPC, S], F32)
        mtmp = sb("mtmp", [BPC, 1], F32)
        stg = sb("stg", [BPC, 8], F32)
        stg0 = sb("stg0", [BPC, 4], F32)

        sy = nc.sync
        vec = nc.vector
        act = nc.scalar

        def chunk_ap(c):
            return pt[:, c * DW : (c + 1) * DW]

        # ------------- sync engine: all DMAs (unrolled, static) ------------
        sy.dma_start(out=M3[:, :], in_=m3eg[:, :]).then_inc(s_in, 16)     # 1
        sy.dma_start(out=xqB[:, :], in_=chunk_ap(0)).then_inc(s_in, 16)   # 2
        sy.dma_start(out=xqA[:, :], in_=chunk_ap(1)).then_inc(s_in, 16)   # 3
        for k in range(2, NCH):
            xqX = xqA if k % 2 else xqB
            sy.wait_ge(s_upc, k - 1)      # nibble-split of c_{k-2} freed xqX
            sy.dma_start(out=xqX[:, :], in_=chunk_ap(k)).then_inc(s_in, 16)
            if k == 2:
                sy.wait_ge(s_free, 1)
                sy.dma_start(out=outd[:, OB : OB + 4], in_=stg0[:, :]).then_inc(
                    s_out, 16
                )
            if k >= 4 and k % 2 == 0:
                i = (k - 4) // 2
                sy.wait_ge(s_free, k - 1)  # vector done iter i
                sy.dma_start(
                    out=outd[:, OB + 4 + 8 * i : OB + 12 + 8 * i], in_=stg[:, :]
                ).then_inc(s_out, 16)
        sy.wait_ge(s_free, NCH)
        sy.dma_start(
            out=outd[:, OB + 4 + 8 * (NIT - 1) : OUTW], in_=stg[:, :]
        ).then_inc(s_out, 16)
        sy.dma_start(out=outd[:, 0 : S + 2], in_=AO[:, :]).then_inc(s_out, 16)
        sy.wait_ge(s_out, 16 * (NIT + 2))

        # --------- activation engine: exp of both nibble planes ------------
        for k in range(NCH):
            act.wait_ge(s_dv, k + 1)
            act.activation(elo[:, :], nlo[:, :], AF.Exp,
                           bias=bcon[:, 0:1], scale=LN2)
            act.activation(ehi[:, :], nhi[:, :], AF.Exp,
                           bias=bcon[:, 0:1], scale=LN2).then_inc(s_da, 1)

        # ---------------- vector engine: decode + the DP -------------------
        def decode(xqX):
            # nibble split (frees xqX), int->f32, zero-masks; exp runs on act
            vec.tensor_scalar(n8l[:, :], xqX[:, :], 15, None, OP.bitwise_and)
            vec.tensor_scalar(n8h[:, :], xqX[:, :], 4, None,
                              OP.logical_shift_right).then_inc(s_upc, 1)
            vec.tensor_copy(nlo[:, :], n8l[:, :])
            vec.tensor_copy(nhi[:, :], n8h[:, :]).then_inc(s_dv, 1)
            vec.tensor_scalar_min(mlo[:, :], nlo[:, :], 1.0)
            vec.tensor_scalar_min(mhi[:, :], nhi[:, :], 1.0)

        LB = (CT // 2) * LW   # label-plane bytes 2048

        def expand(pS, pF):
            # even states <- shared blank (broadcast), odd states <- labels
            vec.tensor_copy(
                bass.AP(pF, 0, [[HW2, BPC], [S, CT // 2], [2, LW + 1]]),
                bass.AP(pS, LB, [[DW, BPC], [1, CT // 2], [0, LW + 1]]))
            vec.tensor_copy(
                bass.AP(pF, 1, [[HW2, BPC], [S, CT // 2], [2, LW]]),
                bass.AP(pS, 0, [[DW, BPC], [LW, CT // 2], [1, LW]]))

        def finish_decode(rDa=None, const_thr=None):
            if rDa is not None:
                vec.wait_ge(s_da, rDa)
            else:
                vec.wait_ge(s_da, const_thr)
            vec.tensor_tensor(pSL[:, :], elo[:, :], mlo[:, :], OP.mult)
            vec.tensor_tensor(pSH[:, :], ehi[:, :], mhi[:, :], OP.mult)
            expand(pSL, pLo)
            expand(pSH, pHi)

        # Deferred renorm: the boundary step (lt%8==7) sums its output row
        # into mtmp via accum_out; the step after (lt%8==0) computes 1/sum
        # into its stage slot; the step after THAT (lt%8==1) folds the scale
        # into its emission multiply. The extra step of deferral keeps every
        # short-op consumer >=4 instructions behind its producer (stale reads
        # of mtmp are harmless: whatever 1/sum value is written is both
        # shipped and applied). The last boundary of the run is neither
        # applied nor shipped.
        def step(src, dst, lt, recip_slot, apply_slot):
            vec.tensor_tensor(a3t[:, :], src[:, 0:S], M3[:, 0:S], OP.mult)
            if recip_slot is not None:
                vec.reciprocal(recip_slot, mtmp[:, :])
            vec.scalar_tensor_tensor(
                s1t[:, :], src[:, 1 : 1 + S], M3[:, S : S + 1],
                src[:, 2 : 2 + S], OP.mult, OP.add,
            )
            vec.tensor_tensor(s2t[:, :], s1t[:, :], a3t[:, :], OP.add)
            pf = pLo if lt % 2 == 0 else pHi
            pslice = pf[:, (lt // 2) * S : (lt // 2 + 1) * S]
            if apply_slot is not None:
                last = vec.scalar_tensor_tensor(
                    dst[:, 2 : 2 + S], s2t[:, :], apply_slot, pslice,
                    OP.mult, OP.mult,
                )
            elif lt % RN == RN - 1:
                last = vec.scalar_tensor_tensor(
                    dst[:, 2 : 2 + S], s2t[:, :], 1.0, pslice,
                    OP.mult, OP.mult, accum_out=mtmp[:, :],
                )
            else:
                last = vec.tensor_tensor(
                    dst[:, 2 : 2 + S], s2t[:, :], pslice, OP.mult
                )
            return last

        # guards stay zero forever; AE body is re-zeroed where the t=0 init
        # does not write; AO body is fully written by the first step.
        vec.memset(AE[:, 0:2], 0.0)
        vec.memset(AO[:, 0:2], 0.0)
        vec.memset(AE[:, 4 : S + 2], 0.0)
        vec.memset(bcon[:, :], -QB * LN2)
        # init factor tiles so a lost pipeline race yields a valid (and
        # consistently shipped+applied) factor instead of uninit garbage;
        # stg0 col 3 stays 1.0 (dummy factor, log == 0)
        vec.memset(stg0[:, :], 1.0)
        vec.memset(stg[:, :], 1.0)
        vec.memset(mtmp[:, :], 1.0)
        vec.wait_ge(s_in, 32)                     # M3 + c0 landed
        decode(xqB)
        finish_decode(const_thr=1)
        vec.tensor_copy(AE[:, 2:4], pLo[:, 0:2])  # t=0 init (tilt pre-baked)
        vec.memset(s2t[:, :], 0.0)   # filler: step 1 must not read AE at gap 0
        last = None
        for lt in range(1, CT):                   # chunk 0: steps 1..31
            src, dst = (AO, AE) if lt % 2 == 0 else (AE, AO)
            rs = stg0[:, lt // RN - 1 : lt // RN] if (
                lt % RN == 0) else None
            aps = stg0[:, lt // RN - 1 : lt // RN] if (
                lt % RN == 1 and lt > 1) else None
            last = step(src, dst, lt, rs, aps)
        last.then_inc(s_free, 1)

        rI = vec.alloc_register("rI")
        rO = vec.alloc_register("rO")
        rD = vec.alloc_register("rD")
        vec.reg_mov(rI, 32)
        vec.reg_mov(rO, 0)
        vec.reg_mov(rD, 1)
        with vec.Fori(0, NIT):
            vec.reg_add(rO, rO, 16)
            vec.wait_ge(s_out, rO)                # stage DMA of prev iter done
            for half, xqX in ((0, xqA), (1, xqB)):
                vec.reg_add(rI, rI, 16)
                vec.wait_ge(s_in, rI)             # this chunk's DMA landed
                decode(xqX)
                vec.reg_add(rD, rD, 1)
                finish_decode(rDa=rD)
                base = 4 * half
                last = None
                for lt in range(CT):
                    src, dst = (AO, AE) if lt % 2 == 0 else (AE, AO)
                    rs = aps = None
                    if lt % RN == 0:
                        c = (base + lt // RN - 1) % 8
                        rs = stg[:, c : c + 1]
                    elif lt % RN == 1:
                        c = (base + lt // RN - 1) % 8
                        aps = stg[:, c : c + 1]
                    last = step(src, dst, lt, rs, aps)
                last.then_inc(s_free, 1)

    return nc


_NC_CACHE = None
_LAST_IN_MAPS = None


def _prep(lp, tg, il, tl):
    """Host-side emission prep. Returns (in_maps, g, shift, sl, ext, m3)."""
    ext = np.zeros((B, S), np.int32)
    ext[:, 1::2] = tg
    prev2 = np.concatenate([np.zeros((B, 2), np.int32), ext[:, :-2]], axis=1)
    m3 = ((ext != 0) & (ext != prev2)).astype(np.float32)
    E = np.take_along_axis(lp, ext[:, None, :], axis=2)      # [B,T,S] f32
    sl = (2 * tl).astype(np.int64)

    nu = sl / il
    g = np.polyval(GCO, nu)
    g = np.clip(g, 0.2, 3.5).astype(np.float64)

    # per-utterance shift so e^{E+c} tops out at 2^(15-QB)
    Emax = E.max(axis=(1, 2)).astype(np.float64)
    shift = (15.0 - QB) * LN2 - Emax

    n4 = np.zeros((B, TP, S), np.uint8)
    for b in range(B):
        ib = int(il[b])
        q = (E[b, :ib].astype(np.float64) + shift[b]) / LN2 + QB
        q[0, 1] += -g[b] / LN2             # tilt on the t=0 init of state 1
        qr = np.round(q)
        n4[b, :ib] = np.where(qr >= 1.0, np.minimum(qr, 15.0), 0.0).astype(
            np.uint8)
        # freeze: blank=1 for all even states, labels=0 -> odd states die at
        # t=il, every even state then freezes (no skip into evens), and
        # A[sl] holds alpha[sl] + e^{-g} alpha[sl-1] = the tilted answer.
        n4[b, ib:, 0] = np.uint8(QB)
    lab = (n4[:, 0::2, 1::2] | (n4[:, 1::2, 1::2] << 4))   # [B,TP/2,128]
    blk = (n4[:, 0::2, 0] | (n4[:, 1::2, 0] << 4))         # [B,TP/2]
    packed = np.concatenate(
        [lab.reshape(B, NCH, (CT // 2) * LW), blk.reshape(B, NCH, CT // 2)],
        axis=2).reshape(B, NCH * DW)
    m3eg = np.zeros((B, S + 1), np.float32)
    m3eg[:, :S] = m3 * np.exp(-2 * g)[:, None]
    m3eg[:, S] = np.exp(-g)

    in_maps = []
    for c in range(NCORES):
        bs = slice(c * BPC, (c + 1) * BPC)
        in_maps.append({
            "pt": np.ascontiguousarray(packed[bs]),
            "m3eg": np.ascontiguousarray(m3eg[bs]),
        })
    return in_maps, g, shift, sl, ext, m3


def _ll_exact(lp, ext, m3, il, sl, bsel):
    """Float64 log-domain DP fallback for utterances in bsel."""
    nb = len(bsel)
    E = np.take_along_axis(
        lp[bsel].astype(np.float64), ext[bsel][:, None, :], axis=2)
    NEGL = -1e30
    a = np.full((nb, S), NEGL)
    a[:, 0] = E[:, 0, 0]
    a[:, 1] = E[:, 0, 1]
    m3b = m3[bsel] > 0
    snap = np.zeros((nb, S))
    ilb = il[bsel]
    for t in range(int(ilb.max())):
        if t > 0:
            a2 = np.concatenate([np.full((nb, 1), NEGL), a[:, :-1]], axis=1)
            a3 = np.where(
                m3b,
                np.concatenate([np.full((nb, 2), NEGL), a[:, :-2]], axis=1),
                NEGL,
            )
            m = np.maximum(np.maximum(a, a2), a3)
            a = m + np.log(
                np.exp(a - m) + np.exp(a2 - m) + np.exp(a3 - m)
            ) + E[:, t, :]
        hit = (ilb - 1) == t
        if hit.any():
            snap[hit] = a[hit]
    slb = sl[bsel]
    r = np.arange(nb)
    return np.logaddexp(snap[r, slb], snap[r, slb - 1])


def kernel(log_probs, targets, input_lengths, target_lengths):
    global _NC_CACHE, _LAST_IN_MAPS
    lp = np.asarray(log_probs, np.float32)
    tg = np.asarray(targets, np.int32)
    il = np.asarray(input_lengths, np.int64)
    tl = np.asarray(target_lengths, np.int64)

    in_maps, g, shift, sl, ext, m3 = _prep(lp, tg, il, tl)
    if _NC_CACHE is None:
        _NC_CACHE = _build_nc()
    _LAST_IN_MAPS = in_maps
    res = run_bass_kernel_spmd(_NC_CACHE, in_maps, core_ids=list(range(NCORES)))

    ll = np.zeros(B, np.float64)
    bad = []
    for b in range(B):
        core, row = b // BPC, b % BPC
        o = res.results[core]["outd"][row].astype(np.float64)
        afin = o[2 + sl[b]]
        rhat = o[OB:OUTW]
        # freeze guarantees afin is the renormed answer mass (order ~1e-6..2);
        # factors are 1/rowsum of a renormed lattice (bounded drift per 8
        # steps). Out-of-range values mean a corrupted run -> exact fallback.
        if (1e-12 < afin < 1e6 and np.all(rhat > 1e-30)
                and np.all(rhat < 1e30)):
            ll[b] = (np.log(afin) - np.log(rhat).sum()
                     - shift[b] * il[b] + g[b] * sl[b])
        else:
            bad.append(b)
    if bad:
        ll[bad] = _ll_exact(lp, ext, m3, il, sl, np.array(bad))
    loss = -ll.sum() / il.sum()
    return np.float32(loss)
